# revision 22
# baseline (speedup 1.0000x reference)
"""Trainium2 Bass kernel for nn_CODEXReconstruction (moe_routing).

Data-parallel over the batch across 8 NeuronCores; all weights replicated.
Per-core pipeline (batch shard B=1024, activations stored transposed
[features, batch] so every layer's weight tensor is used directly as the
matmul stationary operand and no on-device transposes are needed):

    enc1:  h1  = relu(W1.T @ xT + b1)      [512, 1024]
    enc2:  emb = relu(W2.T @ h1 + b2)      [256, 1024]
    experts (t = 0..19):
           ps_t = TW[t].T @ emb + onehot_t.T @ gatemask   (K=20 matmul adds
                  (gate[t,b]-1)*1e30 per column -> relu gates the expert)
           latent += relu(ps_t + Tb[t])    (f32 ping-pong accumulate on DVE)
    dec1/dec2: relu matmuls                [512, 1024]
    dec3:  rec = W3.T @ d2 + b3            [10000, 1024]
           rows <5000: copy out; rows >=5000: softplus + 0.001 via exp/ln

All matmuls run bf16 (f32 PSUM): bf16 keeps the PE's HAM clock gate warm
(fp32r streams never re-warm it and pay 2 cycles/row cold = ~4x slower) and
Fast Weight Load hides LDWEIGHTS. Outputs are written fp16 (10-bit mantissa,
half the store bytes). PSUM tiles are
[128,1024] (2 banks) so each ACT/DVE epilogue op covers a full batch row.
Weight tiles are pre-packed on host so every DMA moves >=2KB per partition
line; constant loads ride the GpSimd queue so the Sync queue starts the
enc1 stream immediately. The gate (incl. the >1-samples-per-treatment rule)
is computed on host from the integer treatment tensor over the FULL batch.
"""

import numpy as np
import ml_dtypes

import bass_rust
import concourse.bass as bass
import concourse.mybir as mybir
import concourse.tile as tile
from concourse.bass_utils import run_bass_kernel_spmd
from concourse.tile import ScopedClock

# ---------------------------------------------------------------------------
# Problem constants (hardcoded per contract)
# ---------------------------------------------------------------------------
IN_F = 5000
IN_FP = 5120                  # zero-padded K so k-tiles are uniform 128
N0, N1, N2 = 512, 512, 256
T = 20
BATCH = 8192
N_CORES = 8
B = BATCH // N_CORES          # 1024 per core
NB = B // 512                 # moving-dim chunks of 512
KP = IN_FP // 256             # 20 packed x/w1 stream steps (2 k-tiles each)
MT_HALF = 40                  # 5000 out-features -> 40 m-tiles (last 8 valid)

F32 = mybir.dt.float32
F16 = mybir.dt.float16
F32R = mybir.dt.float32r
BF16 = mybir.dt.bfloat16
RELU = mybir.ActivationFunctionType.Relu
IDENT = mybir.ActivationFunctionType.Identity
EXP = mybir.ActivationFunctionType.Exp
LN = mybir.ActivationFunctionType.Ln
ADD = mybir.AluOpType.add
MULT = mybir.AluOpType.mult
# softplus(x)+0.001 = ln(C + C*e^x) with C = e^0.001 (this walrus build has no
# Softplus act table; exp/ln/relu/identity all live in one table set)
SP_C = 1.0010005001667084

# ---------------------------------------------------------------------------
# Workaround: this walrus build rejects >1 sync wait per instruction.
# Split extra waits onto injected same-engine NoOps (engine streams are
# in-order, so a preceding same-engine wait is equivalent), and chunk the
# Tile tail-drain's waits across chained drain instructions.
# ---------------------------------------------------------------------------
_uid = [0]


def _nop_with_wait(engine, wait):
    _uid[0] += 1
    nop = mybir.InstNoOp(name=f"WSPLIT-{_uid[0]}", ins=[], outs=[])
    nop.engine = engine
    nop.sync_info = bass_rust.SyncInfo(on_wait=[wait], on_update=[])
    return nop


def split_sync_waits(nc):
    for f in nc.m.functions:
        for bb in f.blocks:
            old = bb.instructions
            if not any(
                i.sync_info and i.sync_info.on_wait and len(i.sync_info.on_wait) > 1
                for i in old
            ):
                continue
            new = []
            for inst in old:
                si = inst.sync_info
                if si is not None and si.on_wait and len(si.on_wait) > 1:
                    waits = list(si.on_wait)
                    for w in waits[:-1]:
                        new.append(_nop_with_wait(inst.engine, w))
                    si.on_wait = [waits[-1]]
                new.append(inst)
            bb.instructions = new


def _patched_drain_and_barrier(self, tick_clock, wait_clock):
    nc = self.nc
    drain_inst = nc.sync.drain()
    wait_clock.add_sem_waits(
        drain_inst.ins, ScopedClock({None: tick_clock.global_clock})
    )
    waits = list(drain_inst.ins.sync_info.on_wait or [])
    if len(waits) > 1:
        drain_inst.ins.sync_info.on_wait = waits[:1]
        for i in range(1, len(waits)):
            extra = nc.sync.drain()
            if extra.ins.sync_info is None:
                extra.ins.sync_info = bass_rust.SyncInfo(
                    on_wait=[waits[i]], on_update=[]
                )
            else:
                extra.ins.sync_info.on_wait = [waits[i]]

    nc.all_engine_barrier()
    assert self.sems is not None
    popped = nc._tile_sem_poison_stack.pop()
    assert popped is self._sem_poison
    nc.clear_and_free_semaphores(list(self.sems.allocated().values()))
    nc.all_engine_barrier()


tile.TileContext._drain_and_barrier = _patched_drain_and_barrier


# ---------------------------------------------------------------------------
# Bass module (one NeuronCore's program; SPMD across 8 cores)
# ---------------------------------------------------------------------------
def build_bass():
    nc = bass.Bass()

    # packed streams: per step j, x holds k-tiles 2j,2j+1 side by side
    xp = nc.dram_tensor("xp", [KP, 128, 2 * B], BF16, kind="ExternalInput")
    w1p = nc.dram_tensor("w1p", [KP, 128, 2 * N0], BF16, kind="ExternalInput")
    w2 = nc.dram_tensor("w2", [N0, N2], BF16, kind="ExternalInput")
    twp = nc.dram_tensor("twp", [T, 128, 2 * N2], BF16, kind="ExternalInput")
    gm = nc.dram_tensor("gm", [T, B], BF16, kind="ExternalInput")
    oh = nc.dram_tensor("oh", [T, T * 128], BF16, kind="ExternalInput")
    dw1 = nc.dram_tensor("dw1", [N2, N1], BF16, kind="ExternalInput")
    dw2 = nc.dram_tensor("dw2", [N1, N0], BF16, kind="ExternalInput")
    # mi-pairs packed: w3*[j, p, mi2*512 + k*128 + c] = W3[k*128+p, (2j+mi2)*128+c]
    w3m = nc.dram_tensor("w3m", [MT_HALF // 2, 128, 1024], BF16, kind="ExternalInput")
    w3v = nc.dram_tensor("w3v", [MT_HALF // 2, 128, 1024], BF16, kind="ExternalInput")
    # bias columns: [128, n_tiles], col j = bias[j*128 : (j+1)*128]
    b1c = nc.dram_tensor("b1c", [128, 4], F32, kind="ExternalInput")
    b2c = nc.dram_tensor("b2c", [128, 2], F32, kind="ExternalInput")
    tbc = nc.dram_tensor("tbc", [128, T * 2], F32, kind="ExternalInput")
    db1c = nc.dram_tensor("db1c", [128, 4], F32, kind="ExternalInput")
    db2c = nc.dram_tensor("db2c", [128, 4], F32, kind="ExternalInput")
    b3mc = nc.dram_tensor("b3mc", [128, MT_HALF], F32, kind="ExternalInput")
    b3vc = nc.dram_tensor("b3vc", [128, MT_HALF], F32, kind="ExternalInput")

    yt = nc.dram_tensor("yt", [2 * IN_F, B], F16, kind="ExternalOutput")

    with tile.TileContext(nc) as tc:
        with (
            tc.tile_pool(name="const", bufs=1) as const,
            tc.tile_pool(name="acts", bufs=8) as acts,
            tc.tile_pool(name="acc", bufs=6) as accp,
            tc.tile_pool(name="xs", bufs=6) as xs,
            tc.tile_pool(name="ws", bufs=6) as wsp,
            tc.tile_pool(name="tws", bufs=6) as tws,
            tc.tile_pool(name="w3s", bufs=4) as w3s,
            tc.tile_pool(name="outs", bufs=3) as outs,
            tc.tile_pool(name="rp", bufs=3) as rp,
            tc.tile_pool(name="ps", bufs=4, space="PSUM") as psp,
        ):
            # ------- HAM warm-up: ~10 dummy matmuls run during the initial
            # DMA latency so the clock gate is at 8/8 when enc1 starts
            warm = const.tile([128, 512], BF16, name="warm")
            nc.vector.memset(warm[:], 0.0)
            wps = psp.tile([128, 512], F32, name="wps", tag="ps")
            for i in range(10):
                nc.tensor.matmul(
                    wps[:], warm[:, :128], warm[:], start=(i == 0), stop=(i == 9)
                )

            # ------- persistent constants (GpSimd queue, off the load path)
            w2_sb = []
            for k in range(4):
                t_ = const.tile([128, N2], BF16, name=f"w2_{k}")
                nc.gpsimd.dma_start(out=t_[:], in_=w2[k * 128:(k + 1) * 128, :])
                w2_sb.append(t_)
            dw1_sb = []
            for k in range(2):
                t_ = const.tile([128, N1], BF16, name=f"dw1_{k}")
                nc.gpsimd.dma_start(out=t_[:], in_=dw1[k * 128:(k + 1) * 128, :])
                dw1_sb.append(t_)
            dw2_sb = []
            for k in range(4):
                t_ = const.tile([128, N0], BF16, name=f"dw2_{k}")
                nc.gpsimd.dma_start(out=t_[:], in_=dw2[k * 128:(k + 1) * 128, :])
                dw2_sb.append(t_)
            gm_sb = const.tile([T, B], BF16, name="gm_sb")
            nc.gpsimd.dma_start(out=gm_sb[:], in_=gm[:])
            oh_sb = const.tile([T, T * 128], BF16, name="oh_sb")
            nc.gpsimd.dma_start(out=oh_sb[:], in_=oh[:])

            def load_bias(name, src, cols):
                t_ = const.tile([128, cols], F32, name=name)
                nc.gpsimd.dma_start(out=t_[:], in_=src[:])
                return t_

            b1_sb = load_bias("b1_sb", b1c, 4)
            b2_sb = load_bias("b2_sb", b2c, 2)
            tb_sb = load_bias("tb_sb", tbc, T * 2)
            db1_sb = load_bias("db1_sb", db1c, 4)
            db2_sb = load_bias("db2_sb", db2c, 4)
            b3m_sb = load_bias("b3m_sb", b3mc, MT_HALF)
            b3v_sb = load_bias("b3v_sb", b3vc, MT_HALF)

            def mk_psum(tag_name):
                # [128, 1024] = 2 PSUM banks; matmuls fill 512-wide halves
                return psp.tile([128, B], F32, name=tag_name, tag="ps")

            # ------- enc1 (bf16): [5120,1024] -> [512,1024]
            h1 = [
                acts.tile([128, B], BF16, name=f"h1_{m}", tag="a1024")
                for m in range(4)
            ]
            ps_h1 = [mk_psum(f"psh1_{m}") for m in range(4)]
            for j in range(KP):
                xk = xs.tile([128, 2 * B], BF16, name=f"x_{j}", tag="x")
                w1k = wsp.tile([128, 2 * N0], BF16, name=f"w1_{j}", tag="w")
                if j == 0:
                    # halves so the s=0 matmuls start after ~half the bytes
                    nc.scalar.dma_start(out=w1k[:, :N0], in_=w1p[j, :, :N0])
                    nc.sync.dma_start(out=xk[:, :B], in_=xp[j, :, :B])
                    nc.scalar.dma_start(out=w1k[:, N0:], in_=w1p[j, :, N0:])
                    nc.sync.dma_start(out=xk[:, B:], in_=xp[j, :, B:])
                else:
                    qa = nc.sync if j % 2 == 0 else nc.scalar
                    qb = nc.scalar if j % 2 == 0 else nc.sync
                    qa.dma_start(out=xk[:], in_=xp[j])
                    qb.dma_start(out=w1k[:], in_=w1p[j])
                for s in range(2):
                    for m in range(4):
                        for n in range(NB):
                            nc.tensor.matmul(
                                ps_h1[m][:, n * 512:(n + 1) * 512],
                                w1k[:, s * N0 + m * 128: s * N0 + (m + 1) * 128],
                                xk[:, s * B + n * 512: s * B + (n + 1) * 512],
                                start=(j == 0 and s == 0),
                                stop=(j == KP - 1 and s == 1),
                            )
            for n in range(NB):
                for m in range(4):
                    sl = slice(n * 512, (n + 1) * 512)
                    nc.scalar.activation(
                        h1[m][:, sl], ps_h1[m][:, sl], RELU, bias=b1_sb[:, m:m + 1]
                    )

            # ------- enc2 (f32r): [512,1024] -> [256,1024]
            emb = [
                acts.tile([128, B], BF16, name=f"emb_{m}", tag="a1024")
                for m in range(2)
            ]
            ps_e = [mk_psum(f"pse_{m}") for m in range(2)]
            for k in range(4):
                for m in range(2):
                    for n in range(NB):
                        nc.tensor.matmul(
                            ps_e[m][:, n * 512:(n + 1) * 512],
                            w2_sb[k][:, m * 128:(m + 1) * 128],
                            h1[k][:, n * 512:(n + 1) * 512],
                            start=(k == 0),
                            stop=(k == 3),
                        )
            for n in range(NB):
                for m in range(2):
                    sl = slice(n * 512, (n + 1) * 512)
                    nc.scalar.activation(
                        emb[m][:, sl], ps_e[m][:, sl], RELU, bias=b2_sb[:, m:m + 1]
                    )

            # ------- experts + gated accumulation (f32r)
            # ping-pong accumulators: out != in0 keeps the DVE add on its
            # fast 2x path (in-place TT falls back to 1x)
            lat = [
                [
                    accp.tile([128, B], F32, name=f"lat_{f}_{p}", tag="lacc")
                    for p in range(2)
                ]
                for f in range(2)
            ]
            for t in range(T):
                twk = tws.tile([128, 2 * N2], BF16, name=f"tw_{t}", tag="tw")
                nc.gpsimd.dma_start(out=twk[:], in_=twp[t])
                for f in range(2):
                    ps = mk_psum(f"pst_{t}_{f}")
                    for k in range(2):
                        for n in range(NB):
                            nc.tensor.matmul(
                                ps[:, n * 512:(n + 1) * 512],
                                twk[:, k * N2 + f * 128: k * N2 + (f + 1) * 128],
                                emb[k][:, n * 512:(n + 1) * 512],
                                start=(k == 0),
                                stop=False,
                            )
                    # += (gate[t,b]-1)*1e30 broadcast over partitions
                    for n in range(NB):
                        nc.tensor.matmul(
                            ps[:, n * 512:(n + 1) * 512],
                            oh_sb[:, t * 128:(t + 1) * 128],
                            gm_sb[:, n * 512:(n + 1) * 512],
                            start=False,
                            stop=True,
                        )
                    bias_ap = tb_sb[:, t * 2 + f:t * 2 + f + 1]
                    if t == 0:
                        nc.scalar.activation(lat[f][0][:], ps[:], RELU, bias=bias_ap)
                    else:
                        r = rp.tile([128, B], F32, name=f"r_{t}_{f}", tag="r")
                        nc.scalar.activation(r[:], ps[:], RELU, bias=bias_ap)
                        # all accumulates on DVE in f32 (GpSimd shares SBUF
                        # ports with DVE; running both slows both ~2.4x)
                        nc.vector.tensor_add(
                            lat[f][t % 2][:], lat[f][(t - 1) % 2][:], r[:]
                        )

            lat_r = [
                accp.tile([128, B], BF16, name=f"latr_{f}", tag="lr")
                for f in range(2)
            ]
            for n in range(NB):
                for f in range(2):
                    sl = slice(n * 512, (n + 1) * 512)
                    nc.vector.tensor_copy(
                        lat_r[f][:, sl], lat[f][(T - 1) % 2][:, sl]
                    )

            # ------- dec1 (f32r): [256,1024] -> [512,1024]
            d1 = [
                acts.tile([128, B], BF16, name=f"d1_{m}", tag="a1024")
                for m in range(4)
            ]
            ps_d1 = [mk_psum(f"psd1_{m}") for m in range(4)]
            for k in range(2):
                for m in range(4):
                    for n in range(NB):
                        nc.tensor.matmul(
                            ps_d1[m][:, n * 512:(n + 1) * 512],
                            dw1_sb[k][:, m * 128:(m + 1) * 128],
                            lat_r[k][:, n * 512:(n + 1) * 512],
                            start=(k == 0),
                            stop=(k == 1),
                        )
            for n in range(NB):
                for m in range(4):
                    sl = slice(n * 512, (n + 1) * 512)
                    nc.scalar.activation(
                        d1[m][:, sl], ps_d1[m][:, sl], RELU, bias=db1_sb[:, m:m + 1]
                    )

            # ------- dec2 (f32r): [512,1024] -> [512,1024], d2 in bf16
            d2 = [
                acts.tile([128, B], BF16, name=f"d2_{m}", tag="a1024")
                for m in range(4)
            ]
            ps_d2 = [mk_psum(f"psd2_{m}") for m in range(4)]
            for k in range(4):
                for m in range(4):
                    for n in range(NB):
                        nc.tensor.matmul(
                            ps_d2[m][:, n * 512:(n + 1) * 512],
                            dw2_sb[k][:, m * 128:(m + 1) * 128],
                            d1[k][:, n * 512:(n + 1) * 512],
                            start=(k == 0),
                            stop=(k == 3),
                        )
            for m in range(4):
                nc.scalar.activation(d2[m][:], ps_d2[m][:], RELU, bias=db2_sb[:, m:m + 1])

            # ------- dec3 (bf16) + output heads, means/vars interleaved per j
            # so the vars half's 2 transcendental ACT passes hide under the
            # means half's PE work. w3 loads and stores move mi-pairs.
            def dec3_pair(wsrc, bias_sb, out_row0, softplus, j):
                w3k = w3s.tile(
                    [128, 1024], BF16, name=f"w3_{out_row0}_{j}", tag="w3"
                )
                nc.gpsimd.dma_start(out=w3k[:], in_=wsrc[j])
                o = outs.tile([128, 2 * B], F16, name=f"o_{out_row0}_{j}", tag="o")
                for mi2 in range(2):
                    mi = 2 * j + mi2
                    mw = 128 if mi < MT_HALF - 1 else (IN_F - 128 * (MT_HALF - 1))
                    ps = mk_psum(f"ps3_{out_row0}_{mi}")
                    for k in range(4):
                        for n in range(NB):
                            nc.tensor.matmul(
                                ps[:, n * 512:(n + 1) * 512],
                                w3k[:, mi2 * 512 + k * 128: mi2 * 512 + (k + 1) * 128],
                                d2[k][:, n * 512:(n + 1) * 512],
                                start=(k == 0),
                                stop=(k == 3),
                            )
                    osl = o[:mw, mi2 * B:(mi2 + 1) * B]
                    bias_ap = bias_sb[:mw, mi:mi + 1]
                    if softplus:
                        sc = rp.tile(
                            [128, B], F32, name=f"sc_{out_row0}_{mi}", tag="sc"
                        )
                        nc.scalar.activation(sc[:mw, :], ps[:mw, :], EXP, bias=bias_ap)
                        nc.vector.tensor_scalar(
                            sc[:mw, :], sc[:mw, :], SP_C, SP_C, op0=MULT, op1=ADD
                        )
                        nc.scalar.activation(osl, sc[:mw, :], LN)
                    else:
                        # means epilogue entirely on DVE (ACT is vars-bound)
                        nc.vector.tensor_scalar_add(osl, ps[:mw, :], bias_ap)
                r0 = out_row0 + 2 * j * 128
                if j < MT_HALF // 2 - 1:
                    # both mi full: one DMA writes 256 DRAM rows
                    nc.sync.dma_start(
                        out=yt[r0:r0 + 256, :].rearrange("(t p) b -> p t b", p=128),
                        in_=o.rearrange("p (t b) -> p t b", t=2),
                    )
                else:
                    nc.sync.dma_start(out=yt[r0:r0 + 128, :], in_=o[:, :B])
                    tail = IN_F - 128 * (MT_HALF - 1)
                    nc.sync.dma_start(
                        out=yt[r0 + 128:r0 + 128 + tail, :],
                        in_=o[:tail, B:],
                    )

            for j in range(MT_HALF // 2):
                dec3_pair(w3v, b3v_sb, IN_F, True, j)
                dec3_pair(w3m, b3m_sb, 0, False, j)

    split_sync_waits(nc)
    return nc


# ---------------------------------------------------------------------------
# Host glue
# ---------------------------------------------------------------------------
_NC_CACHE = []


def _get_nc():
    if not _NC_CACHE:
        _NC_CACHE.append(build_bass())
    return _NC_CACHE[0]


def _bias_cols(b, ntiles):
    """[D] -> [128, ntiles]; col j = b[j*128:(j+1)*128], zero-padded."""
    out = np.zeros((128, ntiles), np.float32)
    b = np.asarray(b, np.float32)
    for j in range(ntiles):
        seg = b[j * 128:min((j + 1) * 128, b.shape[0])]
        out[: seg.shape[0], j] = seg
    return out


def _prep_shared(inputs):
    f32 = lambda a: np.ascontiguousarray(np.asarray(a), dtype=np.float32)
    bf16 = ml_dtypes.bfloat16
    w1 = f32(inputs["enc_W1"])
    w2 = f32(inputs["enc_W2"])
    tw = f32(inputs["T_W"])
    dw1 = f32(inputs["dec_W1"])
    dw2 = f32(inputs["dec_W2"])
    w3 = f32(inputs["dec_W3"])

    # w1 zero-padded to [5120, 512], packed in pairs of k-tiles:
    # w1p[j, p, s*512 + c] = W1[(2j+s)*128 + p, c]
    w1z = np.zeros((IN_FP, N0), np.float32)
    w1z[:IN_F] = w1
    w1p = np.ascontiguousarray(
        w1z.reshape(KP, 2, 128, N0).transpose(0, 2, 1, 3).reshape(KP, 128, 2 * N0)
    ).astype(bf16)

    # T_W packed: twp[t, p, k*256 + c] = T_W[t, k*128 + p, c]
    twp = np.ascontiguousarray(
        tw.reshape(T, 2, 128, N2).transpose(0, 2, 1, 3).reshape(T, 128, 2 * N2)
    ).astype(bf16)

    # dec_W3 halves packed in mi-pairs:
    # w3p[j, p, mi2*512 + k*128 + c] = W3[k*128 + p, (2j+mi2)*128 + c]
    def tile_w3(cols):
        out = np.zeros((MT_HALF // 2, 128, 1024), np.float32)
        for k in range(4):
            blk = cols[k * 128:(k + 1) * 128, :]          # [128, <=5120]
            cw = blk.shape[1]
            padded = np.zeros((128, MT_HALF * 128), np.float32)
            padded[:, :cw] = blk
            per_mi = padded.reshape(128, MT_HALF, 128).transpose(1, 0, 2)
            for mi2 in range(2):
                out[:, :, mi2 * 512 + k * 128: mi2 * 512 + (k + 1) * 128] = (
                    per_mi[mi2::2]
                )
        return np.ascontiguousarray(out).astype(bf16)

    w3m = tile_w3(w3[:, :IN_F])
    w3v = tile_w3(w3[:, IN_F:])

    # gate over the FULL batch (apply_t uses full-batch counts)
    treat = np.asarray(inputs["treatment"])
    tvals = np.arange(1, T + 1)
    mask = (treat[:, None, :] == tvals[None, :, None]).any(-1)  # [B, T]
    apply_t = mask.sum(0) > 1
    gate = (mask & apply_t[None, :]).astype(np.float32)         # [B, T]
    gm_full = np.ascontiguousarray((gate.T - 1.0) * 1e30)       # [T, B]

    oh = np.zeros((T, T * 128), np.float32)
    for t in range(T):
        oh[t, t * 128:(t + 1) * 128] = 1.0

    shared = {
        "w1p": w1p,
        "w2": w2.astype(bf16),
        "twp": twp,
        "oh": oh.astype(bf16),
        "dw1": dw1.astype(bf16),
        "dw2": dw2.astype(bf16),
        "w3m": w3m,
        "w3v": w3v,
        "b1c": _bias_cols(inputs["enc_b1"], 4),
        "b2c": _bias_cols(inputs["enc_b2"], 2),
        "tbc": np.ascontiguousarray(
            np.asarray(inputs["T_b"], dtype=np.float32)
            .reshape(T, 2, 128)
            .transpose(2, 0, 1)
            .reshape(128, T * 2)
        ),
        "db1c": _bias_cols(inputs["dec_b1"], 4),
        "db2c": _bias_cols(inputs["dec_b2"], 4),
        "b3mc": _bias_cols(np.asarray(inputs["dec_b3"])[:IN_F], MT_HALF),
        "b3vc": _bias_cols(np.asarray(inputs["dec_b3"])[IN_F:], MT_HALF),
    }
    x = f32(inputs["input"])
    in_maps = []
    for c in range(N_CORES):
        m = dict(shared)
        # xT zero-padded to [5120, B], packed in pairs of k-tiles:
        # xp[j, p, s*B + c] = xT[(2j+s)*128 + p, c]
        xt = np.zeros((IN_FP, B), np.float32)
        xt[:IN_F] = x[c * B:(c + 1) * B, :].T
        m["xp"] = np.ascontiguousarray(
            xt.reshape(KP, 2, 128, B).transpose(0, 2, 1, 3).reshape(KP, 128, 2 * B)
        ).astype(bf16)
        m["gm"] = np.ascontiguousarray(gm_full[:, c * B:(c + 1) * B]).astype(bf16)
        in_maps.append(m)
    return in_maps


def kernel(**inputs) -> np.ndarray:
    nc = _get_nc()
    in_maps = _prep_shared(inputs)
    res = run_bass_kernel_spmd(nc, in_maps, core_ids=list(range(N_CORES)))
    out = np.empty((BATCH, 2 * IN_F), np.float32)
    for c in range(N_CORES):
        out[c * B:(c + 1) * B, :] = res.results[c]["yt"].T.astype(np.float32)
    return out
